# revision 1
# baseline (speedup 1.0000x reference)
"""Trainium2 Bass kernel for the 3-layer GAT (nn_GAT_24326694764623).

Strategy (8 NeuronCores, SPMD):
  - Nodes sharded 6250/core. Edges (incl. self loops) sorted by dst and
    assigned to the dst's core, packed into 128-edge chunks such that no
    dst's edge run crosses a chunk and no chunk crosses a 128-node tile.
  - Per layer: AllGather node table [h | al_s | al_d] (f32), indirect-DMA
    gather per chunk by src, per-dst softmax via an edge-vs-edge equality
    matrix (sel) + one-hot matmuls, aggregation in h-space (aggregate-then-
    transform), transform GEMM per 128-node tile, global BatchNorm via a
    tiny AllReduce, relu + residual.
  - Attention logits (al_s/al_d) are computed on the vector engine to keep
    the softmax input exact fp32.
"""
import os
import sys
import hashlib

for _p in ("/opt/trn_rl_repo", "/root/.axon_site/_ro/trn_rl_repo"):
    if _p not in sys.path:
        sys.path.insert(0, _p)

import numpy as np

_PHASE = int(os.environ.get("GAT_PHASE", "9"))
DBG_MARKS = {}
N, E = 50000, 400000
HEADS, C, HID, LAYERS = 4, 128, 128, 3
NCORES = 8
NB = N // NCORES            # 6250 real nodes per core
P = 128
NT = (NB + P - 1) // P      # 49 node tiles per core
NBP = NT * P                # 6272 padded rows per core
NTAB = NCORES * NBP         # 50176 global table rows
DCOL = HID + 2 * HEADS      # 136 table cols [h | al_s | al_d]
NEG = 0.2
EPS = 1e-5
GRP = 4                     # chunks per instruction group

_cache = {}


def _build_plan(edge_index):
    src = np.concatenate([edge_index[0].astype(np.int64), np.arange(N, dtype=np.int64)])
    dst = np.concatenate([edge_index[1].astype(np.int64), np.arange(N, dtype=np.int64)])
    isloop = np.concatenate([np.zeros(E, bool), np.ones(N, bool)])
    order = np.argsort(dst, kind="stable")
    src, dst, isloop = src[order], dst[order], isloop[order]

    per = []
    for c in range(NCORES):
        m = (dst >= c * NB) & (dst < (c + 1) * NB)
        per.append((src[m], dst[m] - c * NB, isloop[m]))

    deg = np.zeros((NCORES, NB), np.int64)
    for c in range(NCORES):
        np.add.at(deg[c], per[c][1], 1)

    bases, nds = [], []
    b = 0
    while b < NB:
        tile_end = min((b // P + 1) * P, NB)
        nd = 0
        cnt = np.zeros(NCORES, np.int64)
        while b + nd < tile_end:
            c2 = cnt + deg[:, b + nd]
            if c2.max() > P:
                break
            cnt = c2
            nd += 1
        assert nd > 0, "single dst exceeds 128 edges on some core"
        bases.append(b)
        nds.append(nd)
        b += nd
    nch = len(bases)
    ngrp = (nch + GRP - 1) // GRP
    nchp = ngrp * GRP
    bases += [0] * (nchp - nch)
    nds += [0] * (nchp - nch)

    srci = np.zeros((NCORES, nchp, P), np.int32)
    statg = np.zeros((NCORES, ngrp, P, 3 * GRP), np.float32)
    statg[:, :, :, GRP:2 * GRP] = -1e30  # pad bias default: all slots dead
    for c in range(NCORES):
        s, dl, il = per[c]
        ptr = 0
        for k in range(nch):
            bb, nd = bases[k], nds[k]
            g, j = k // GRP, k % GRP
            ne = int(deg[c, bb:bb + nd].sum())
            sl = slice(ptr, ptr + ne)
            glob = (s[sl] // NB) * NBP + (s[sl] % NB)
            srci[c, k, :ne] = glob.astype(np.int32)
            dloc = (dl[sl] - (bb // P) * P).astype(np.float32)  # tile-local dst
            col = np.zeros(P, np.float32)
            col[:ne] = dloc
            col[ne:] = bb % P
            statg[c, g, :, j] = col
            pb = np.full(P, -1e30, np.float32)
            pb[:ne] = 0.0
            statg[c, g, :, GRP + j] = pb
            sf = np.zeros(P, np.float32)
            sf[:ne] = il[sl].astype(np.float32)
            statg[c, g, :, 2 * GRP + j] = sf
            ptr += ne
        assert ptr == len(s)
    dstrow = np.ascontiguousarray(
        statg[:, :, :, :GRP].transpose(0, 1, 3, 2).reshape(NCORES, ngrp, GRP * P))
    meta = [(bases[k] // P, bases[k] % P, nds[k]) for k in range(nchp)]
    return nch, nchp, ngrp, meta, srci, statg, dstrow


def _prep_weights(inp):
    f32 = np.float32
    Wc1 = inp["Wc"][:, :32].astype(f32)
    Wc2 = inp["Wc"][:, 32:].astype(f32)
    W0 = np.concatenate([Wc1, Wc2 @ inp["Wf"].astype(f32)], axis=1)  # (128, 48)
    b0 = inp["bc"].astype(f32) + Wc2 @ inp["bf"].astype(f32)
    vsd = np.zeros((LAYERS, 2 * HEADS, HID), f32)
    Wm = np.zeros((LAYERS, HEADS * C, HID), f32)
    for l in range(LAYERS):
        W = inp["Wl"][l].astype(f32).reshape(HEADS, C, HID)
        for hh in range(HEADS):
            vsd[l, hh] = W[hh].T @ inp["a_src"][l, hh].astype(f32)
            vsd[l, HEADS + hh] = W[hh].T @ inp["a_dst"][l, hh].astype(f32)
            Wm[l, hh * C:(hh + 1) * C, :] = W[hh].T / HEADS
    gbcol = np.zeros((LAYERS, P, 2), f32)
    gbcol[:, :, 0] = inp["gamma"].astype(f32)
    gbcol[:, :, 1] = inp["beta"].astype(f32)
    return (W0.T.copy(), b0[None, :].copy(), vsd, Wm, gbcol,
            inp["Wout"].astype(f32).copy(), np.array([[inp["bout"][0]]], f32))


def _build_nc(nch, nchp, ngrp, meta):
    import concourse.bass as bass
    import concourse.bacc as bacc
    import concourse.mybir as mybir
    import concourse.tile as tile
    import bass_rust as _br
    from contextlib import ExitStack
    from concourse.masks import make_identity

    f32 = mybir.dt.float32
    i32 = mybir.dt.int32
    Alu = mybir.AluOpType
    Act = mybir.ActivationFunctionType

    nc = bacc.Bacc(None, target_bir_lowering=False)
    feat48 = nc.declare_dram_parameter("feat48", [48, NBP], f32, isOutput=False)
    statg_d = nc.declare_dram_parameter("statg", [ngrp, P, 3 * GRP], f32, isOutput=False)
    dstrow_d = nc.declare_dram_parameter("dstrow", [ngrp, GRP * P], f32, isOutput=False)
    srci_d = nc.declare_dram_parameter("srci", [nchp, P], i32, isOutput=False)
    W0T_d = nc.declare_dram_parameter("W0T", [48, P], f32, isOutput=False)
    b0_d = nc.declare_dram_parameter("b0row", [1, P], f32, isOutput=False)
    vsd_d = nc.declare_dram_parameter("vsd", [LAYERS, 2 * HEADS, HID], f32, isOutput=False)
    Wm_d = nc.declare_dram_parameter("Wm3", [LAYERS, HEADS * C, HID], f32, isOutput=False)
    gb_d = nc.declare_dram_parameter("gbcol", [LAYERS, P, 2], f32, isOutput=False)
    wout_d = nc.declare_dram_parameter("woutrow", [1, P], f32, isOutput=False)
    bout_d = nc.declare_dram_parameter("boutsc", [1, 1], f32, isOutput=False)
    out_d = nc.declare_dram_parameter("outp", [NBP, 1], f32, isOutput=True)

    with ExitStack() as ctx:
        tc = ctx.enter_context(tile.TileContext(nc))
        const = ctx.enter_context(tc.tile_pool(name="const", bufs=1))
        big = ctx.enter_context(tc.tile_pool(name="big", bufs=1))
        sb = ctx.enter_context(tc.tile_pool(name="sb", bufs=3))
        gpool = ctx.enter_context(tc.tile_pool(name="gp", bufs=3))
        idxp = ctx.enter_context(tc.tile_pool(name="idxp", bufs=8))
        hpool = ctx.enter_context(tc.tile_pool(name="hp", bufs=3))
        ps_agg = ctx.enter_context(tc.tile_pool(name="ps_agg", bufs=3, space="PSUM"))
        ps_sm = ctx.enter_context(tc.tile_pool(name="ps_sm", bufs=2, space="PSUM"))
        ps_den = ctx.enter_context(tc.tile_pool(name="ps_den", bufs=2, space="PSUM"))
        ps_out = ctx.enter_context(tc.tile_pool(name="ps_out", bufs=1, space="PSUM"))
        dram = ctx.enter_context(tc.tile_pool(name="dram", bufs=1, space="DRAM"))

        agin = dram.tile([NBP, DCOL], f32)
        tables = [dram.tile([NTAB, DCOL], f32, addr_space="Shared", name=f"table{i}")
                  for i in range(LAYERS)]
        bnins = [dram.tile([P, 2], f32, name=f"bnin{i}") for i in range(LAYERS)]
        bnouts = [dram.tile([P, 2], f32, addr_space="Shared", name=f"bnout{i}")
                  for i in range(LAYERS)]
        rowbuf = dram.tile([P, 2], f32)

        # ---------------- constants ----------------
        iota_row = const.tile([P, P], f32)
        nc.gpsimd.iota(iota_row[:], pattern=[[1, P]], base=0, channel_multiplier=0,
                       allow_small_or_imprecise_dtypes=True)
        ident = const.tile([P, P], f32)
        make_identity(nc, ident[:])
        ones_col = const.tile([P, 1], f32)
        nc.vector.memset(ones_col[:], 1.0)
        rows3 = const.tile([1, 3 * P], f32)
        nc.sync.dma_start(out=rows3[:, 0:P], in_=b0_d[:1, :])
        nc.sync.dma_start(out=rows3[:, P:2 * P], in_=wout_d[:1, :])
        nc.sync.dma_start(out=rows3[:, 2 * P:2 * P + 1], in_=bout_d[:1, :1])
        b0_bc = const.tile([P, P], f32)
        nc.gpsimd.partition_broadcast(b0_bc[:], rows3[:1, 0:P])
        wout_bc = const.tile([P, P], f32)
        nc.gpsimd.partition_broadcast(wout_bc[:], rows3[:1, P:2 * P])
        bout_col = const.tile([P, 1], f32)
        nc.gpsimd.partition_broadcast(bout_col[:], rows3[:1, 2 * P:2 * P + 1])
        W0T_t = const.tile([48, P], f32)
        nc.sync.dma_start(out=W0T_t[:], in_=W0T_d[:])

        # two big ping-pong node-feature buffers (SBUF-resident)
        hbufA = big.tile([P, NT * P], f32)   # (128, 6272)
        hbufB = big.tile([P, NT * P], f32)

        def vsd_bcast_tiles(l):
            vrow = sb.tile([1, 2 * HEADS * P], f32, tag="vsdrow")
            nc.sync.dma_start(out=vrow[:], in_=vsd_d[l, :, :].rearrange("a b -> (a b)")[None, :])
            vt = sb.tile([P, 2 * HEADS * P], f32, tag="vsdbc")
            for j in range(2 * HEADS):
                nc.gpsimd.partition_broadcast(vt[:, j * P:(j + 1) * P],
                                              vrow[:1, j * P:(j + 1) * P])
            return [vt[:, j * P:(j + 1) * P] for j in range(2 * HEADS)]

        agin_writes = []

        def alsd_and_agin(hbuf, t, vsd_ts, scratch_tag):
            """al_s/al_d for node tile t from hbuf, write [h|al_s|al_d] to agin."""
            hsl = hbuf[:, t * P:(t + 1) * P]
            comb = sb.tile([P, DCOL], f32, tag="comb")
            nc.scalar.copy(out=comb[:, 0:HID], in_=hsl)
            scr = sb.tile([P, P], f32, tag=scratch_tag)
            for j in range(2 * HEADS):
                i_mul = nc.vector.tensor_tensor(out=scr[:], in0=hsl, in1=vsd_ts[j], op=Alu.mult)
                i_red = nc.vector.tensor_reduce(out=comb[:, HID + j:HID + j + 1], in_=scr[:],
                                        op=Alu.add, axis=mybir.AxisListType.X)
                DBG_MARKS[f"mul_{id(hbuf)}_t{t}_j{j}"] = i_mul.ins.name
                DBG_MARKS[f"red_{id(hbuf)}_t{t}_j{j}"] = i_red.ins.name
            r0 = t * P
            d1 = nc.sync.dma_start(out=agin[r0:r0 + P, :], in_=comb[:])
            DBG_MARKS[f"combdma_{id(hbuf)}_t{t}"] = d1.ins.name
            agin_writes.append(d1)

        # ---------------- encoder ----------------
        vsd0 = vsd_bcast_tiles(0)
        for t in range(NT):
            lhs48 = sb.tile([48, P], f32, tag="lhs48")
            nc.sync.dma_start(out=lhs48[:], in_=feat48[:, t * P:(t + 1) * P])
            pse = ps_out.tile([P, P], f32, space="PSUM", tag="psout")
            nc.tensor.matmul(out=pse[:], lhsT=lhs48[:], rhs=W0T_t[:], start=True, stop=True)
            hsl = hbufA[:, t * P:(t + 1) * P]
            nc.vector.tensor_tensor(out=hsl, in0=pse[:], in1=b0_bc[:], op=Alu.add)
            nc.vector.tensor_scalar_max(out=hsl, in0=hsl, scalar1=0.0)
            alsd_and_agin(hbufA, t, vsd0, "scr_enc")
        cc = nc.gpsimd.collective_compute(
            "AllGather", Alu.bypass, replica_groups=[list(range(NCORES))],
            ins=[agin.opt()], outs=[tables[0].opt()])
        for d in agin_writes:
            _br.add_dep_helper(cc.ins, d.ins, sync=True, reason="AG after agin writes")
        agin_writes.clear()

        if _PHASE == 0:
            for t in range(NT):
                nc.sync.dma_start(out=out_d[t * P:(t + 1) * P, :], in_=hbufA[:, t * P:t * P + 1])
        # ---------------- layers ----------------
        for l in range(LAYERS if _PHASE > 0 else 0):
            hprev = hbufA if l % 2 == 0 else hbufB
            hpre = hbufB if l % 2 == 0 else hbufA
            wm_t = sb.tile([P, HEADS * P], f32, tag="wm")
            for j in range(HEADS):
                nc.sync.dma_start(out=wm_t[:, j * P:(j + 1) * P],
                                  in_=Wm_d[l, j * P:(j + 1) * P, :])
            Wm_ts = [wm_t[:, j * P:(j + 1) * P] for j in range(HEADS)]
            gb_t = sb.tile([P, 2], f32, tag="gb")
            nc.sync.dma_start(out=gb_t[:], in_=gb_d[l, :, :])
            if _PHASE == 40 + l:
                dtmp = sb.tile([P, NT], f32, tag="dtmp")
                dd = nc.sync.dma_start(out=dtmp[:], in_=agin[0:NBP, int(os.environ.get('GAT_COL', '0'))].rearrange("(t p) -> p t", p=P))
                for d_ in agin_writes:
                    _br.add_dep_helper(dd.ins, d_.ins, sync=True, reason="dump after agin writes")
                for t in range(NT):
                    nc.sync.dma_start(out=out_d[t * P:(t + 1) * P, :], in_=dtmp[:, t:t + 1])
                break
            if _PHASE == 10 + l:
                dtmp = sb.tile([P, NT], f32, tag="dtmp")
                dd = nc.sync.dma_start(out=dtmp[:], in_=tables[l][0:NBP, int(os.environ.get('GAT_COL', '0'))].rearrange("(t p) -> p t", p=P))
                _br.add_dep_helper(dd.ins, cc.ins, sync=True, reason="dump after AG")
                for t in range(NT):
                    nc.sync.dma_start(out=out_d[t * P:(t + 1) * P, :], in_=dtmp[:, t:t + 1])
                break

            last_gather = None
            psB_tile = None
            psD_tile = None
            cur_tile = -1
            done_tiles = []

            def finish_tile(t):
                # normalize by den (PSUM reads), transpose, transform GEMM, stats
                rden = sb.tile([P, HEADS], f32, tag="rden")
                nc.vector.tensor_scalar_add(out=rden[:], in0=psD_tile[:], scalar1=1e-16)
                nc.vector.reciprocal(out=rden[:], in_=rden[:])
                h_agg = hpool.tile([P, HEADS * P], f32, tag="hagg")
                nc.vector.tensor_tensor(
                    out=h_agg[:].rearrange("p (h k) -> p h k", h=HEADS),
                    in0=psB_tile[:].rearrange("p (h k) -> p h k", h=HEADS),
                    in1=rden[:, :, None].to_broadcast([P, HEADS, P]),
                    op=Alu.mult)
                pst = ps_agg.tile([P, HEADS * P], f32, space="PSUM", tag="psB")
                for j in range(HEADS):
                    nc.tensor.transpose(out=pst[:, j * P:(j + 1) * P],
                                        in_=h_agg[:, j * P:(j + 1) * P], identity=ident[:])
                aggT = sb.tile([P, HEADS * P], f32, tag="aggT")
                nc.scalar.copy(out=aggT[:], in_=pst[:])
                pso = ps_out.tile([P, P], f32, space="PSUM", tag="psout")
                for j in range(HEADS):
                    nc.tensor.matmul(out=pso[:], lhsT=aggT[:, j * P:(j + 1) * P],
                                     rhs=Wm_ts[j], start=(j == 0), stop=(j == HEADS - 1))
                hsl = hpre[:, t * P:(t + 1) * P]
                nc.scalar.copy(out=hsl, in_=pso[:])
                nrow = min(P, NB - t * P)
                sq = sb.tile([P, P], f32, tag="sq")
                nc.vector.tensor_tensor(out=sq[:nrow, :], in0=hsl[:nrow, :],
                                        in1=hsl[:nrow, :], op=Alu.mult)
                stp = ps_sm.tile([P, 2 * GRP * HEADS], f32, space="PSUM", tag="psAD")
                nc.tensor.matmul(out=stp[:, 0:1], lhsT=hsl[:nrow, :],
                                 rhs=ones_col[:nrow, :], start=True, stop=True)
                nc.tensor.matmul(out=stp[:, 1:2], lhsT=sq[:nrow, :],
                                 rhs=ones_col[:nrow, :], start=True, stop=True)
                if t == 0:
                    nc.vector.tensor_copy(out=stats_sb[:], in_=stp[:, 0:2])
                else:
                    nc.vector.tensor_tensor(out=stats_sb[:], in0=stats_sb[:],
                                            in1=stp[:, 0:2], op=Alu.add)

            stats_sb = sb.tile([P, 2], f32, tag="statsb")

            for g in range(ngrp):
                stat_t = sb.tile([P, 3 * GRP], f32, tag="stat")
                nc.sync.dma_start(out=stat_t[:], in_=statg_d[g, :, :])
                drow_t = sb.tile([1, GRP * P], f32, tag="drow")
                nc.sync.dma_start(out=drow_t[:], in_=dstrow_d[g:g + 1, :])
                dstb = sb.tile([P, GRP * P], f32, tag="dstb")
                nc.gpsimd.partition_broadcast(dstb[:], drow_t[:1, :])
                sel_g = sb.tile([P, GRP * P], f32, tag="selg")
                nc.vector.tensor_tensor(
                    out=sel_g[:].rearrange("p (j e) -> p j e", j=GRP),
                    in0=stat_t[:, 0:GRP, None].to_broadcast([P, GRP, P]),
                    in1=dstb[:].rearrange("p (j e) -> p j e", j=GRP),
                    op=Alu.is_equal)
                m01_g = sb.tile([P, GRP * P], f32, tag="m01g")
                nc.vector.tensor_tensor(
                    out=m01_g[:].rearrange("p (j e) -> p j e", j=GRP),
                    in0=stat_t[:, 0:GRP, None].to_broadcast([P, GRP, P]),
                    in1=iota_row[:, None, :].to_broadcast([P, GRP, P]),
                    op=Alu.is_equal)

                G_g = gpool.tile([P, GRP * DCOL], f32, tag="G")
                gi_list = []
                for j in range(GRP):
                    k = g * GRP + j
                    sidx = idxp.tile([P, 1], i32, tag="sidx")
                    nc.sync.dma_start(out=sidx[:], in_=srci_d[k, :, None])
                    gi = nc.gpsimd.indirect_dma_start(
                        out=G_g[:, j * DCOL:(j + 1) * DCOL], out_offset=None,
                        in_=tables[l][:],
                        in_offset=bass.IndirectOffsetOnAxis(ap=sidx[:, :1], axis=0))
                    _br.add_dep_helper(gi.ins, cc.ins, sync=True, reason="gather after AG")
                    gi_list.append(gi)
                last_gather = gi_list[-1]

                # rhsA = al_d at self edges (G cols 132:136 * selfflag)
                rhsA = sb.tile([P, GRP * HEADS], f32, tag="rhsA")
                nc.vector.tensor_tensor(
                    out=rhsA[:].rearrange("p (j h) -> p j h", j=GRP),
                    in0=G_g[:].rearrange("p (j c) -> p j c", j=GRP)[:, :, HID + HEADS:DCOL],
                    in1=stat_t[:, 2 * GRP:3 * GRP, None].to_broadcast([P, GRP, HEADS]),
                    op=Alu.mult)
                psAD = ps_sm.tile([P, 2 * GRP * HEADS], f32, space="PSUM", tag="psAD")
                psA = psAD[:, 0:GRP * HEADS]
                psD = psAD[:, GRP * HEADS:2 * GRP * HEADS]
                for j in range(GRP):
                    nc.tensor.matmul(out=psA[:, j * HEADS:(j + 1) * HEADS],
                                     lhsT=sel_g[:, j * P:(j + 1) * P],
                                     rhs=rhsA[:, j * HEADS:(j + 1) * HEADS],
                                     start=True, stop=True)
                ea = sb.tile([P, GRP * HEADS], f32, tag="ea")
                nc.vector.tensor_tensor(
                    out=ea[:].rearrange("p (j h) -> p j h", j=GRP),
                    in0=G_g[:].rearrange("p (j c) -> p j c", j=GRP)[:, :, HID:HID + HEADS],
                    in1=psA.rearrange("p (j h) -> p j h", j=GRP),
                    op=Alu.add)
                nc.vector.scalar_tensor_tensor(out=ea[:], in0=ea[:], scalar=NEG,
                                               in1=ea[:], op0=Alu.mult, op1=Alu.max)
                nc.vector.tensor_tensor(
                    out=ea[:].rearrange("p (j h) -> p j h", j=GRP),
                    in0=ea[:].rearrange("p (j h) -> p j h", j=GRP),
                    in1=stat_t[:, GRP:2 * GRP, None].to_broadcast([P, GRP, HEADS]),
                    op=Alu.add)
                ex = sb.tile([P, GRP * HEADS], f32, tag="ex")
                nc.scalar.activation(out=ex[:], in_=ea[:], func=Act.Exp)

                for j in range(GRP):
                    k = g * GRP + j
                    tk, bl, nd = meta[k]
                    if nd == 0:
                        continue
                    if tk != cur_tile:
                        if cur_tile >= 0:
                            finish_tile(cur_tile)
                            done_tiles.append(cur_tile)
                        cur_tile = tk
                        psB_tile = ps_agg.tile([P, HEADS * P], f32, space="PSUM", tag="psB")
                        psD_tile = ps_den.tile([P, HEADS], f32, space="PSUM", tag="psD")
                        first = True
                    else:
                        first = False
                    last = (k == nch - 1) or (meta[k + 1][0] != tk)
                    Hs = sb.tile([P, HEADS * P], f32, tag="Hs")
                    for hh in range(HEADS):
                        nc.scalar.activation(
                            out=Hs[:, hh * P:(hh + 1) * P],
                            in_=G_g[:, j * DCOL:j * DCOL + HID],
                            func=Act.Copy, scale=ex[:, j * HEADS + hh:j * HEADS + hh + 1])
                    nc.tensor.matmul(out=psB_tile[:], lhsT=m01_g[:, j * P:(j + 1) * P],
                                     rhs=Hs[:], start=first, stop=last)
                    nc.tensor.matmul(out=psD_tile[:],
                                     lhsT=m01_g[:, j * P:(j + 1) * P],
                                     rhs=ex[:, j * HEADS:(j + 1) * HEADS],
                                     start=first, stop=last)
            finish_tile(cur_tile)
            done_tiles.append(cur_tile)
            assert sorted(done_tiles) == list(range(NT)), done_tiles
            if _PHASE == 2 * l + 1:
                for t in range(NT):
                    nc.sync.dma_start(out=out_d[t * P:(t + 1) * P, :], in_=hpre[:, t * P:t * P + 1])
                break

            # ---- BN stats -> AllReduce -> scale/shift rows ----
            d_bn = nc.sync.dma_start(out=bnins[l][:, :], in_=stats_sb[:])
            ar = nc.gpsimd.collective_compute(
                "AllReduce", Alu.add, replica_groups=[list(range(NCORES))],
                ins=[bnins[l].opt()], outs=[bnouts[l].opt()])
            _br.add_dep_helper(ar.ins, d_bn.ins, sync=True, reason="AR after stats write")
            st2 = sb.tile([P, 2], f32, tag="st2")
            d_ar = nc.sync.dma_start(out=st2[:], in_=bnouts[l][:, :])
            _br.add_dep_helper(d_ar.ins, ar.ins, sync=True, reason="read after AR")
            mu = sb.tile([P, 1], f32, tag="mu")
            nc.vector.tensor_scalar_mul(out=mu[:], in0=st2[:, 0:1], scalar1=1.0 / N)
            var = sb.tile([P, 1], f32, tag="var")
            nc.vector.tensor_scalar_mul(out=var[:], in0=st2[:, 1:2], scalar1=1.0 / N)
            musq = sb.tile([P, 1], f32, tag="musq")
            nc.vector.tensor_tensor(out=musq[:], in0=mu[:], in1=mu[:], op=Alu.mult)
            nc.vector.tensor_tensor(out=var[:], in0=var[:], in1=musq[:], op=Alu.subtract)
            nc.vector.tensor_scalar_add(out=var[:], in0=var[:], scalar1=EPS)
            rstd = sb.tile([P, 1], f32, tag="rstd")
            nc.scalar.sqrt(out=rstd[:], in_=var[:])
            nc.vector.reciprocal(out=rstd[:], in_=rstd[:])
            ssc = sb.tile([P, 2], f32, tag="ssc")  # [scale | shift] columns
            nc.vector.tensor_tensor(out=ssc[:, 0:1], in0=gb_t[:, 0:1], in1=rstd[:], op=Alu.mult)
            nc.vector.tensor_tensor(out=musq[:], in0=mu[:], in1=ssc[:, 0:1], op=Alu.mult)
            nc.vector.tensor_tensor(out=ssc[:, 1:2], in0=gb_t[:, 1:2], in1=musq[:], op=Alu.subtract)
            # cols -> rows via DRAM bounce (engines cannot shift partitions)
            nc.sync.dma_start(out=rowbuf[:, :], in_=ssc[:])
            srow = sb.tile([1, P], f32, tag="srow")
            nc.sync.dma_start(out=srow[:], in_=rowbuf[:, 0:1].rearrange("p c -> c p"))
            hrow = sb.tile([1, P], f32, tag="hrow")
            nc.sync.dma_start(out=hrow[:], in_=rowbuf[:, 1:2].rearrange("p c -> c p"))
            scale_bc = sb.tile([P, P], f32, tag="scalebc")
            nc.gpsimd.partition_broadcast(scale_bc[:], srow[:1, :])
            shift_bc = sb.tile([P, P], f32, tag="shiftbc")
            nc.gpsimd.partition_broadcast(shift_bc[:], hrow[:1, :])

            # ---- apply BN + relu + residual (+ next-layer table / final head) ----
            vsd_n = vsd_bcast_tiles(l + 1) if l + 1 < LAYERS else None
            if _PHASE == 50 and l == 0:
                vt0 = vsd_n[0].tensor  # underlying tile
                for t in range(NT):
                    nc.sync.dma_start(out=out_d[t * P:(t + 1) * P, :], in_=vsd_n[0][:, t:t + 1])
                break
            for t in range(NT):
                hsl = hpre[:, t * P:(t + 1) * P]
                hpv = hprev[:, t * P:(t + 1) * P]
                tmp = sb.tile([P, P], f32, tag="applytmp")
                nc.vector.tensor_tensor(out=tmp[:], in0=hsl, in1=scale_bc[:], op=Alu.mult)
                nc.vector.tensor_tensor(out=tmp[:], in0=tmp[:], in1=shift_bc[:], op=Alu.add)
                nc.vector.tensor_scalar_max(out=tmp[:], in0=tmp[:], scalar1=0.0)
                i_add = nc.vector.tensor_tensor(out=hsl, in0=tmp[:], in1=hpv, op=Alu.add)
                DBG_MARKS[f"apply_add_l{l}_t{t}"] = i_add.ins.name
                if l + 1 < LAYERS:
                    alsd_and_agin(hpre, t, vsd_n, "scr_l")
                else:
                    scr = sb.tile([P, P], f32, tag="scr_f")
                    ocol = sb.tile([P, 1], f32, tag="ocol")
                    nc.vector.tensor_tensor(out=scr[:], in0=hsl, in1=wout_bc[:], op=Alu.mult)
                    nc.vector.tensor_reduce(out=ocol[:], in_=scr[:],
                                            op=Alu.add, axis=mybir.AxisListType.X)
                    nc.vector.tensor_tensor(out=ocol[:], in0=ocol[:], in1=bout_col[:], op=Alu.add)
                    nc.vector.tensor_scalar_min(out=ocol[:], in0=ocol[:], scalar1=10.0)
                    nc.vector.tensor_scalar_max(out=ocol[:], in0=ocol[:], scalar1=-10.0)
                    nc.sync.dma_start(out=out_d[t * P:(t + 1) * P, :], in_=ocol[:])
            if _PHASE == 2 * l + 2:
                for t in range(NT):
                    nc.sync.dma_start(out=out_d[t * P:(t + 1) * P, :], in_=hpre[:, t * P:t * P + 1])
                break
            if l + 1 < LAYERS:
                cc = nc.gpsimd.collective_compute(
                    "AllGather", Alu.bypass, replica_groups=[list(range(NCORES))],
                    ins=[agin.opt()], outs=[tables[l + 1].opt()])
                _br.add_dep_helper(cc.ins, last_gather.ins, sync=True, reason="AG after gathers")
                for d in agin_writes:
                    _br.add_dep_helper(cc.ins, d.ins, sync=True, reason="AG after agin writes")
                agin_writes.clear()

    nc.compile()
    return nc


def _get_compiled(edge_index):
    key = hashlib.md5(np.ascontiguousarray(edge_index).tobytes()).hexdigest()
    if key not in _cache:
        plan = _build_plan(edge_index)
        nch, nchp, ngrp, meta, srci, statg, dstrow = plan
        nc = _build_nc(nch, nchp, ngrp, meta)
        _cache[key] = (nc, srci, statg, dstrow)
    return _cache[key]


def _make_in_maps(inputs, srci, statg, dstrow):
    W0T, b0row, vsd, Wm3, gbcol, woutrow, boutsc = _prep_weights(inputs)
    x = inputs["x"].astype(np.float32)
    emb = inputs["emb"].astype(np.float32)
    in_maps = []
    for c in range(NCORES):
        f48 = np.zeros((48, NBP), np.float32)
        blk = slice(c * NB, (c + 1) * NB)
        f48[:32, :NB] = emb[blk].T
        f48[32:, :NB] = x[blk].T
        in_maps.append({
            "feat48": f48, "statg": statg[c], "dstrow": dstrow[c],
            "srci": srci[c], "W0T": W0T, "b0row": b0row, "vsd": vsd,
            "Wm3": Wm3, "gbcol": gbcol, "woutrow": woutrow, "boutsc": boutsc,
        })
    return in_maps


def kernel(**inputs):
    from concourse.bass_utils import run_bass_kernel_spmd
    nc, srci, statg, dstrow = _get_compiled(np.asarray(inputs["edge_index"]))
    in_maps = _make_in_maps(inputs, srci, statg, dstrow)
    res = run_bass_kernel_spmd(nc, in_maps, list(range(NCORES)))
    out = np.concatenate([res.results[c]["outp"][:NB] for c in range(NCORES)], axis=0)
    return out.astype(np.float32)

